# revision 21
# baseline (speedup 1.0000x reference)
"""Trainium2 Bass kernel for nn_AttentionLayer (segment softmax attention pooling).

Computation (reference):
    h = tanh(x @ W1 + b1)            # [N, A]
    s = h @ W2 + b2                  # [N, 1]
    per-segment softmax over s, out[b] = sum_i softmax_w_i * x_i   # [B, D]

Strategy:
  - Shard the N=500k instances across 8 NeuronCores (data parallel), weights
    replicated. Host pre-transposes x so each core streams xT [D=128, rows]
    tiles with fully contiguous DMA and D on partitions.
  - Per core, one pass over x:
      PE:  hT = W1^T @ xT            (W1 stationary, rows stream)
      ACT: th = tanh(hT + b1)
      PE:  sbc = W2rep^T @ th        (score broadcast to all 128 partitions)
      ACT: ebc = exp(sbc + b2)
      DVE: tensor_tensor_reduce(xT * ebc) summed per fixed 1024-row window
  - Device outputs: per-window weighted sums WACC [D, nwin] and the raw e row
    (e values for every instance).  All segment logic is on the host: window
    sums for windows fully inside one segment are used directly; windows that
    contain a segment boundary are recomputed on the host from x and the
    exported e (a few hundred small dot products).  Denominators come from
    bincount over the exported e.  exp() without max-subtraction is safe here
    (scores are O(+-5)), and numerator/denominator use identical e values.
"""

import numpy as np

# Problem constants (hardcoded per contract; kernel.py must be self-contained).
N = 500_000
D = 128
A = 128
B = 256
NCORES = 8
RPC = N // NCORES            # rows per core = 62500
CHUNK = 2048                 # rows per streamed tile
WIN = 1024                   # rows per reduction window
NCHUNK = -(-RPC // CHUNK)    # 31
RPAD = NCHUNK * CHUNK        # 63488
NWIN = RPAD // WIN           # 62
MM_N = 512                   # fp32 moving-operand max free dim

_prog_cache = {}


def _build_program(b2val: float):
    import concourse.bacc as bacc
    from concourse import mybir
    from concourse.tile import TileContext

    f32 = mybir.dt.float32
    f32r = mybir.dt.float32r
    nc = bacc.Bacc("TRN2", target_bir_lowering=False, debug=False,
                   num_devices=NCORES)

    # Declared f32r so the load is a plain (cast-free, HWDGE-eligible) copy
    # and the matmul consumer passes BIR verification; the bytes are ordinary
    # fp32 (np float32 maps to both), and the reduce path bitcasts back to
    # f32 so the numerator sees full-precision x.
    xt = nc.dram_tensor("xt", [D, RPAD], f32r, kind="ExternalInput")
    w1 = nc.dram_tensor("w1", [D, A], f32, kind="ExternalInput")
    w2r = nc.dram_tensor("w2r", [A, 128], f32, kind="ExternalInput")
    b1 = nc.dram_tensor("b1", [A, 1], f32, kind="ExternalInput")
    b2 = nc.dram_tensor("b2", [128, 1], f32, kind="ExternalInput")
    wacc = nc.dram_tensor("wacc", [D, NWIN], f32, kind="ExternalOutput")
    eout = nc.dram_tensor("eout", [NCHUNK, CHUNK], f32, kind="ExternalOutput")

    with TileContext(nc) as tc:
        with tc.tile_pool(name="const", bufs=1) as cpool, \
             tc.tile_pool(name="xtp", bufs=3) as xpool, \
             tc.tile_pool(name="thp", bufs=2) as thpool, \
             tc.tile_pool(name="ebp", bufs=3) as ebpool, \
             tc.tile_pool(name="junkp", bufs=2) as jpool, \
             tc.tile_pool(name="accp", bufs=1) as apool, \
             tc.tile_pool(name="psh", bufs=2, space="PSUM") as psh, \
             tc.tile_pool(name="pss", bufs=1, space="PSUM") as pss:

            # f32r (reduced-mantissa fp32) runs the PE at 1 cycle/row vs 4 for
            # fp32. SWDGE DMA casts f32->f32r during the load for free.
            w1sb = cpool.tile([D, A], f32r, tag="w1")
            w2rsb = cpool.tile([A, 128], f32r, tag="w2r")
            b1sb = cpool.tile([A, 1], f32, tag="b1")
            b2sb = cpool.tile([128, 1], f32, tag="b2")
            nc.gpsimd.dma_start(out=w1sb[:], in_=w1[:])
            nc.gpsimd.dma_start(out=w2rsb[:], in_=w2r[:])
            nc.sync.dma_start(out=b1sb[:], in_=b1[:])
            nc.sync.dma_start(out=b2sb[:], in_=b2[:])

            waccsb = apool.tile([D, NWIN], f32, tag="wacc")
            nc.vector.memset(waccsb[:], 0.0)

            # Software pipeline: stage 1 (load + h-matmuls + tanh) for chunk c
            # is emitted in the same iteration as stage 2 (score matmuls +
            # exp + reduce) for chunk c-1, so the scheduler's program-order
            # priority keeps every engine fed: PE prefers next-chunk h-matmuls
            # over current-chunk score-matmuls, which keeps ACT gap-free.
            HALF = CHUNK // 2
            xtiles, ths = {}, {}
            for c in range(NCHUNK + 1):
                if c < NCHUNK:
                    xtile = xpool.tile([D, CHUNK], f32r, tag="x")
                    nc.gpsimd.dma_start(out=xtile[:],
                                        in_=xt[:, c * CHUNK:(c + 1) * CHUNK])
                    xtiles[c] = xtile
                    th = thpool.tile([A, CHUNK], f32r, tag="th")
                    ths[c] = th
                    for h in range(2):
                        hps = psh.tile([A, HALF], f32, tag="hps")
                        for i in range(HALF // MM_N):
                            off = h * HALF + i * MM_N
                            nc.tensor.matmul(
                                out=hps[:, i * MM_N:(i + 1) * MM_N],
                                lhsT=w1sb[:],
                                rhs=xtile[:, off:off + MM_N],
                                start=True, stop=True)
                        nc.scalar.activation(
                            out=th[:, h * HALF:(h + 1) * HALF],
                            in_=hps[:],
                            func=mybir.ActivationFunctionType.Tanh,
                            bias=b1sb[:, 0:1])
                if c >= 1:
                    p = c - 1
                    xtile_p, th_p = xtiles.pop(p), ths.pop(p)
                    sps = pss.tile([128, CHUNK], f32, tag="sps")
                    for i in range(CHUNK // MM_N):
                        nc.tensor.matmul(out=sps[:, i * MM_N:(i + 1) * MM_N],
                                         lhsT=w2rsb[:],
                                         rhs=th_p[:, i * MM_N:(i + 1) * MM_N],
                                         start=True, stop=True)
                    eb = ebpool.tile([128, CHUNK], f32, tag="eb")
                    nc.scalar.activation(out=eb[:], in_=sps[:],
                                         func=mybir.ActivationFunctionType.Exp,
                                         bias=b2sb[:, 0:1])
                    for w in range(CHUNK // WIN):
                        gw = p * (CHUNK // WIN) + w
                        junk = jpool.tile([D, 1], f32, tag="junk")
                        nc.vector.affine_mul_reduce(
                            out=junk[:].to_broadcast([D, WIN]),
                            accum_out=waccsb[:, gw:gw + 1],
                            in0=xtile_p[:, w * WIN:(w + 1) * WIN].bitcast(f32),
                            in1=eb[:, w * WIN:(w + 1) * WIN],
                            scale=1.0,
                            bias=0.0)
                    nc.sync.dma_start(out=eout[p:p + 1, :], in_=eb[0:1, :])

            nc.sync.dma_start(out=wacc[:], in_=waccsb[:])

    nc.compile()
    return nc


def _run_device(xt_shards, W1, W2, b1, b2, trace=False):
    from concourse.bass_utils import run_bass_kernel_spmd

    key = float(b2)
    if key not in _prog_cache:
        _prog_cache[key] = _build_program(key)
    nc = _prog_cache[key]

    w1_in = np.ascontiguousarray(W1, dtype=np.float32)
    w2r_in = np.ascontiguousarray(np.tile(W2.reshape(A, 1), (1, 128)),
                                  dtype=np.float32)
    b1_in = np.ascontiguousarray(b1.reshape(A, 1), dtype=np.float32)
    b2_in = np.full((128, 1), np.float32(b2), dtype=np.float32)

    in_maps = [{"xt": xt_shards[i], "w1": w1_in, "w2r": w2r_in, "b1": b1_in,
                "b2": b2_in}
               for i in range(NCORES)]
    res = run_bass_kernel_spmd(nc, in_maps, core_ids=list(range(NCORES)),
                               trace=trace)
    return res


def kernel(x, batch_index, W1, b1, W2, b2, _want_results=False, _trace=False):
    x = np.ascontiguousarray(np.asarray(x, dtype=np.float32))
    bi = np.asarray(batch_index)
    bi64 = bi.astype(np.int64)
    W1 = np.asarray(W1, dtype=np.float32)
    b1 = np.asarray(b1, dtype=np.float32)
    W2 = np.asarray(W2, dtype=np.float32)
    b2v = float(np.asarray(b2, dtype=np.float32).reshape(-1)[0])

    assert x.shape == (N, D)

    # Host pre-transpose: xT [D, N], then per-core zero-padded shards.
    xtf = np.ascontiguousarray(x.T)
    xt_shards = []
    for i in range(NCORES):
        sh = np.zeros((D, RPAD), dtype=np.float32)
        sh[:, :RPC] = xtf[:, i * RPC:(i + 1) * RPC]
        xt_shards.append(sh)

    res = _run_device(xt_shards, W1, W2, b1, b2v, trace=_trace)

    # Gather device outputs.
    e = np.empty(N, dtype=np.float32)
    waccs = []
    for i in range(NCORES):
        e[i * RPC:(i + 1) * RPC] = \
            res.results[i]["eout"].reshape(-1)[:RPC]
        waccs.append(res.results[i]["wacc"])

    # Denominators: segment sums of e (same values the device used).
    denom = np.bincount(bi64, weights=e.astype(np.float64), minlength=B)

    # Numerators: pure windows from device sums; boundary windows recomputed.
    num = np.zeros((B, D), dtype=np.float64)
    for i in range(NCORES):
        wacc_i = waccs[i]
        base = i * RPC
        for w in range(NWIN):
            glo = base + w * WIN
            if glo >= base + RPC:
                break
            ghi = min(glo + WIN, base + RPC)
            b_first = bi64[glo]
            b_last = bi64[ghi - 1]
            if b_first == b_last:
                # Window entirely in one segment (zero-pad rows contribute 0).
                num[b_first] += wacc_i[:, w]
            else:
                sub = bi64[glo:ghi]
                cuts = np.flatnonzero(np.diff(sub)) + 1
                bounds = np.concatenate(([0], cuts, [ghi - glo]))
                for k in range(len(bounds) - 1):
                    lo, hi = glo + bounds[k], glo + bounds[k + 1]
                    num[sub[bounds[k]]] += \
                        e[lo:hi].astype(np.float64) @ x[lo:hi].astype(np.float64)

    dn = denom[:, None]
    out = np.divide(num, dn, out=np.zeros_like(num), where=dn > 0)
    out = out.astype(np.float32)
    if _want_results:
        return out, res
    return out


# revision 22
# speedup vs baseline: 1.1122x; 1.1122x over previous
"""Trainium2 Bass kernel for nn_AttentionLayer (segment softmax attention pooling).

Computation (reference):
    h = tanh(x @ W1 + b1)            # [N, A]
    s = h @ W2 + b2                  # [N, 1]
    per-segment softmax over s, out[b] = sum_i softmax_w_i * x_i   # [B, D]

Strategy:
  - Shard the N=500k instances across 8 NeuronCores (data parallel), weights
    replicated. Host pre-transposes x so each core streams xT [D=128, rows]
    tiles with fully contiguous DMA and D on partitions.
  - Per core, one pass over x:
      PE:  hT = W1^T @ xT            (W1 stationary, rows stream)
      ACT: th = tanh(hT + b1)
      PE:  sbc = W2rep^T @ th        (score broadcast to all 128 partitions)
      ACT: ebc = exp(sbc + b2)
      DVE: tensor_tensor_reduce(xT * ebc) summed per fixed 1024-row window
  - Device outputs: per-window weighted sums WACC [D, nwin] and the raw e row
    (e values for every instance).  All segment logic is on the host: window
    sums for windows fully inside one segment are used directly; windows that
    contain a segment boundary are recomputed on the host from x and the
    exported e (a few hundred small dot products).  Denominators come from
    bincount over the exported e.  exp() without max-subtraction is safe here
    (scores are O(+-5)), and numerator/denominator use identical e values.
"""

import numpy as np

# Problem constants (hardcoded per contract; kernel.py must be self-contained).
N = 500_000
D = 128
A = 128
B = 256
NCORES = 8
RPC = N // NCORES            # rows per core = 62500
CHUNK = 2048                 # rows per streamed tile
WIN = 1024                   # rows per reduction window
NCHUNK = -(-RPC // CHUNK)    # 31
RPAD = NCHUNK * CHUNK        # 63488
NWIN = RPAD // WIN           # 62
MM_N = 512                   # fp32 moving-operand max free dim

_prog_cache = {}


def _build_program(b2val: float):
    import concourse.bacc as bacc
    from concourse import mybir
    from concourse.tile import TileContext

    f32 = mybir.dt.float32
    f32r = mybir.dt.float32r
    nc = bacc.Bacc("TRN2", target_bir_lowering=False, debug=False,
                   num_devices=NCORES)

    # Declared f32r so the load is a plain (cast-free, HWDGE-eligible) copy
    # and the matmul consumer passes BIR verification; the bytes are ordinary
    # fp32 (np float32 maps to both), and the reduce path bitcasts back to
    # f32 so the numerator sees full-precision x.
    xt = nc.dram_tensor("xt", [D, RPAD], f32r, kind="ExternalInput")
    w1 = nc.dram_tensor("w1", [D, A], f32, kind="ExternalInput")
    w2r = nc.dram_tensor("w2r", [A, 128], f32, kind="ExternalInput")
    b1 = nc.dram_tensor("b1", [A, 1], f32, kind="ExternalInput")
    b2 = nc.dram_tensor("b2", [128, 1], f32, kind="ExternalInput")
    wacc = nc.dram_tensor("wacc", [D, NWIN], f32, kind="ExternalOutput")
    eout = nc.dram_tensor("eout", [NCHUNK, CHUNK], f32, kind="ExternalOutput")

    with TileContext(nc) as tc:
        with tc.tile_pool(name="const", bufs=1) as cpool, \
             tc.tile_pool(name="xtp", bufs=3) as xpool, \
             tc.tile_pool(name="thp", bufs=2) as thpool, \
             tc.tile_pool(name="ebp", bufs=3) as ebpool, \
             tc.tile_pool(name="junkp", bufs=2) as jpool, \
             tc.tile_pool(name="accp", bufs=1) as apool, \
             tc.tile_pool(name="psh", bufs=2, space="PSUM") as psh, \
             tc.tile_pool(name="pss", bufs=1, space="PSUM") as pss:

            # f32r (reduced-mantissa fp32) runs the PE at 1 cycle/row vs 4 for
            # fp32. SWDGE DMA casts f32->f32r during the load for free.
            w1sb = cpool.tile([D, A], f32r, tag="w1")
            w2rsb = cpool.tile([A, 128], f32r, tag="w2r")
            b1sb = cpool.tile([A, 1], f32, tag="b1")
            b2sb = cpool.tile([128, 1], f32, tag="b2")
            nc.gpsimd.dma_start(out=w1sb[:], in_=w1[:])
            nc.gpsimd.dma_start(out=w2rsb[:], in_=w2r[:])
            nc.sync.dma_start(out=b1sb[:], in_=b1[:])
            nc.sync.dma_start(out=b2sb[:], in_=b2[:])

            waccsb = apool.tile([D, NWIN], f32, tag="wacc")
            nc.vector.memset(waccsb[:], 0.0)

            # Software pipeline: stage 1 (load + h-matmuls + tanh) for chunk c
            # is emitted in the same iteration as stage 2 (score matmuls +
            # exp + reduce) for chunk c-1, so the scheduler's program-order
            # priority keeps every engine fed: PE prefers next-chunk h-matmuls
            # over current-chunk score-matmuls, which keeps ACT gap-free.
            HALF = CHUNK // 2
            xtiles, ths = {}, {}

            def load_chunk(c):
                xtile = xpool.tile([D, CHUNK], f32r, tag="x")
                nc.gpsimd.dma_start(out=xtile[:],
                                    in_=xt[:, c * CHUNK:(c + 1) * CHUNK])
                xtiles[c] = xtile

            load_chunk(0)
            if NCHUNK > 1:
                load_chunk(1)
            for c in range(NCHUNK + 1):
                if c + 2 < NCHUNK:
                    load_chunk(c + 2)
                if c < NCHUNK:
                    xtile = xtiles[c]
                    th = thpool.tile([A, CHUNK], f32r, tag="th")
                    ths[c] = th
                    for h in range(2):
                        hps = psh.tile([A, HALF], f32, tag="hps")
                        for i in range(HALF // MM_N):
                            off = h * HALF + i * MM_N
                            nc.tensor.matmul(
                                out=hps[:, i * MM_N:(i + 1) * MM_N],
                                lhsT=w1sb[:],
                                rhs=xtile[:, off:off + MM_N],
                                start=True, stop=True)
                        nc.scalar.activation(
                            out=th[:, h * HALF:(h + 1) * HALF],
                            in_=hps[:],
                            func=mybir.ActivationFunctionType.Tanh,
                            bias=b1sb[:, 0:1])
                if c >= 1:
                    p = c - 1
                    xtile_p, th_p = xtiles.pop(p), ths.pop(p)
                    sps = pss.tile([128, CHUNK], f32, tag="sps")
                    for i in range(CHUNK // MM_N):
                        nc.tensor.matmul(out=sps[:, i * MM_N:(i + 1) * MM_N],
                                         lhsT=w2rsb[:],
                                         rhs=th_p[:, i * MM_N:(i + 1) * MM_N],
                                         start=True, stop=True)
                    eb = ebpool.tile([128, CHUNK], f32, tag="eb")
                    nc.scalar.activation(out=eb[:], in_=sps[:],
                                         func=mybir.ActivationFunctionType.Exp,
                                         bias=b2sb[:, 0:1])
                    for w in range(CHUNK // WIN):
                        gw = p * (CHUNK // WIN) + w
                        junk = jpool.tile([D, 1], f32, tag="junk")
                        nc.vector.affine_mul_reduce(
                            out=junk[:].to_broadcast([D, WIN]),
                            accum_out=waccsb[:, gw:gw + 1],
                            in0=xtile_p[:, w * WIN:(w + 1) * WIN].bitcast(f32),
                            in1=eb[:, w * WIN:(w + 1) * WIN],
                            scale=1.0,
                            bias=0.0)
                    nc.sync.dma_start(out=eout[p:p + 1, :], in_=eb[0:1, :])

            nc.sync.dma_start(out=wacc[:], in_=waccsb[:])

    nc.compile()
    return nc


def _run_device(xt_shards, W1, W2, b1, b2, trace=False):
    from concourse.bass_utils import run_bass_kernel_spmd

    key = float(b2)
    if key not in _prog_cache:
        _prog_cache[key] = _build_program(key)
    nc = _prog_cache[key]

    w1_in = np.ascontiguousarray(W1, dtype=np.float32)
    w2r_in = np.ascontiguousarray(np.tile(W2.reshape(A, 1), (1, 128)),
                                  dtype=np.float32)
    b1_in = np.ascontiguousarray(b1.reshape(A, 1), dtype=np.float32)
    b2_in = np.full((128, 1), np.float32(b2), dtype=np.float32)

    in_maps = [{"xt": xt_shards[i], "w1": w1_in, "w2r": w2r_in, "b1": b1_in,
                "b2": b2_in}
               for i in range(NCORES)]
    res = run_bass_kernel_spmd(nc, in_maps, core_ids=list(range(NCORES)),
                               trace=trace)
    return res


def kernel(x, batch_index, W1, b1, W2, b2, _want_results=False, _trace=False):
    x = np.ascontiguousarray(np.asarray(x, dtype=np.float32))
    bi = np.asarray(batch_index)
    bi64 = bi.astype(np.int64)
    W1 = np.asarray(W1, dtype=np.float32)
    b1 = np.asarray(b1, dtype=np.float32)
    W2 = np.asarray(W2, dtype=np.float32)
    b2v = float(np.asarray(b2, dtype=np.float32).reshape(-1)[0])

    assert x.shape == (N, D)

    # Host pre-transpose: xT [D, N], then per-core zero-padded shards.
    xtf = np.ascontiguousarray(x.T)
    xt_shards = []
    for i in range(NCORES):
        sh = np.zeros((D, RPAD), dtype=np.float32)
        sh[:, :RPC] = xtf[:, i * RPC:(i + 1) * RPC]
        xt_shards.append(sh)

    res = _run_device(xt_shards, W1, W2, b1, b2v, trace=_trace)

    # Gather device outputs.
    e = np.empty(N, dtype=np.float32)
    waccs = []
    for i in range(NCORES):
        e[i * RPC:(i + 1) * RPC] = \
            res.results[i]["eout"].reshape(-1)[:RPC]
        waccs.append(res.results[i]["wacc"])

    # Denominators: segment sums of e (same values the device used).
    denom = np.bincount(bi64, weights=e.astype(np.float64), minlength=B)

    # Numerators: pure windows from device sums; boundary windows recomputed.
    num = np.zeros((B, D), dtype=np.float64)
    for i in range(NCORES):
        wacc_i = waccs[i]
        base = i * RPC
        for w in range(NWIN):
            glo = base + w * WIN
            if glo >= base + RPC:
                break
            ghi = min(glo + WIN, base + RPC)
            b_first = bi64[glo]
            b_last = bi64[ghi - 1]
            if b_first == b_last:
                # Window entirely in one segment (zero-pad rows contribute 0).
                num[b_first] += wacc_i[:, w]
            else:
                sub = bi64[glo:ghi]
                cuts = np.flatnonzero(np.diff(sub)) + 1
                bounds = np.concatenate(([0], cuts, [ghi - glo]))
                for k in range(len(bounds) - 1):
                    lo, hi = glo + bounds[k], glo + bounds[k + 1]
                    num[sub[bounds[k]]] += \
                        e[lo:hi].astype(np.float64) @ x[lo:hi].astype(np.float64)

    dn = denom[:, None]
    out = np.divide(num, dn, out=np.zeros_like(num), where=dn > 0)
    out = out.astype(np.float32)
    if _want_results:
        return out, res
    return out
